# revision 34
# baseline (speedup 1.0000x reference)
"""Masked multi-head attention on 8 TRN2 NeuronCores.

Sharding: core = (batch b, head-group hg). Each core computes the attention
output for one batch element and 4 of the 8 heads (a 256-wide column slice
of E). Rows with mask==0 are dropped host-side: masked queries produce
all-zero output rows, and masked keys are excluded by zeroing both their v
rows (x rows are zero) and the denominator ones-column, so no exp bias is
needed. bq is folded into etype_emb host-side; bk drops out of softmax
(per-query constant shift); bv is added host-side after the divide.

Everything bf16 on chip except PSUM accumulation and the f32 output.

Per-core pipeline, ACT(exp)-bound by design:
  qT/kT = W.T @ xT          (E-cols on partitions, S free; ete added to qT)
  v     = xT.T @ Wv         (S on partitions, DH free) + masked ones column
  per head h, kc-group g (3 key chunks), q-chunk (<=512):
    sT   = kT.T @ qT  into psum [128, 3, 512]  (keys on partitions)
    att  = exp(sT/8)  one wide ACT op per (h, q-chunk, g)
    hT[j] += att[:, j128].T @ v_aug   (transposed PV: 65-wide, full PE util)
  out[h] = hT (+den col) DMA'd per head; host transposes and divides
"""

import os

import ml_dtypes
import numpy as np

import concourse.bacc as bacc
import concourse.tile as tile
from concourse import mybir
from concourse.bass_utils import run_bass_kernel_spmd

BF = mybir.dt.bfloat16
F32 = mybir.dt.float32
F32R = mybir.dt.float32r  # f32 storage, single-pass PE mode (full rate at N>=256)

B, S, F, E, H = 4, 2048, 512, 512, 8
DH = 64
NCORES = 8
HPC = 4            # heads per core
CPC = HPC * DH     # output columns per core

LAST_RESULT = None  # BassKernelResults of the most recent run (for test harness)


def _qchunks(SPL):
    out, off = [], 0
    while off < SPL:
        ln = min(512, SPL - off)
        out.append((off, ln))
        off += ln
    return out


def _offsets(SP):
    NKC = SP // 128
    WV_OFF = 0
    WK_OFF = 1024
    WQ_OFF = 2048
    VM_OFF = 3072                    # [128, NKC*HPC] pad-aware v ones-column
    XT_OFF = VM_OFF + NKC * HPC
    ETE_OFF = XT_OFF + 4 * SP
    COLS = ETE_OFF + 2 * SP
    return WV_OFF, WK_OFF, WQ_OFF, VM_OFF, XT_OFF, ETE_OFF, COLS


def _build(SP, loop_reps=None, abl="full", SPL=None):
    if SPL is None:
        SPL = SP
    NKC = SP // 128
    NQC = (SPL + 127) // 128
    WV_OFF, WK_OFF, WQ_OFF, VM_OFF, XT_OFF, ETE_OFF, COLS = _offsets(SP)

    nc = bacc.Bacc()
    blob = nc.declare_dram_parameter("blob", [128, COLS], BF, isOutput=False)
    outp = nc.declare_dram_parameter("out", [HPC, 65, NQC * 128], F32, isOutput=True)

    with tile.TileContext(nc) as tc:
        with (
            tc.tile_pool(name="sing", bufs=1) as sing,
            tc.tile_pool(name="hsb", bufs=2) as hsb_pool,
            tc.tile_pool(name="attp", bufs=12) as attp,
            tc.tile_pool(name="ps", bufs=2, space="PSUM") as ps,
        ):
            def _body():
                _emit(nc, SP, SPL, NKC, NQC, WV_OFF, WK_OFF, WQ_OFF, VM_OFF,
                      XT_OFF, ETE_OFF, COLS, blob, outp,
                      sing, hsb_pool, attp, ps, abl, looped=loop_reps is not None)

            if loop_reps is None:
                _body()
            else:
                with tc.For_i(0, loop_reps, 1):
                    _body()
    nc.compile()
    return nc


def _xt_moving(bsb, XT_OFF, SP, f, qoff, qlen):
    """Moving-operand APs over the kc-major xT layout for q range [qoff, qoff+qlen)."""
    view = bsb[:, XT_OFF:XT_OFF + 4 * SP].rearrange("p (kc f c) -> p kc f c", f=4, c=128)
    out = []
    kc0, nfull, rem = qoff // 128, qlen // 128, qlen % 128
    if nfull:
        out.append((0, nfull * 128, view[:, kc0:kc0 + nfull, f, :]))
    if rem:
        out.append((nfull * 128, rem, view[:, kc0 + nfull, f, :rem]))
    return out


def _emit(nc, SP, SPL, NKC, NQC, WV_OFF, WK_OFF, WQ_OFF, VM_OFF, XT_OFF,
          ETE_OFF, COLS, blob, outp, sing, hsb_pool, attp, ps, abl="full",
          looped=False):
    QCH = _qchunks(SPL)     # q chunks (proj + scores)
    # k-proj chunks: 256 wide, aligned with the 2-kc xT DMA groups so a
    # scores unit never waits on xT beyond its own key range.
    QCH_K = []
    off = 0
    while off < SP:
        ln = min(256, SP - off)
        QCH_K.append((off, ln))
        off += ln
    KCG = [(i, min(i + 3, NKC)) for i in range(0, NKC, 3)]
    NG = len(KCG)

    NB = 2 if looped else 1
    bsb = sing.tile([128, COLS], BF, bufs=NB, name="bsb")
    KGS = 2  # kc chunks per xT DMA group
    KG = [(i, min(i + KGS, NKC)) for i in range(0, NKC, KGS)]

    # Input DMA over three queues (SP + ACT HWDGE rings, gpsimd SWDGE),
    # ordered so the attention front (WK/WQ/ete0/early xT) lands first.
    def xt_dma(eng, gi):
        k0, k1 = KG[gi]
        c0, c1 = XT_OFF + k0 * 512, XT_OFF + k1 * 512
        eng.dma_start(out=bsb[:, c0:c1], in_=blob[:, c0:c1])

    # Global issue order puts the scores-critical front (WK, WQ, ete[:512],
    # xT g0/g1) first; the model serializes DMA engines, so issue order is
    # landing order.
    E0A = min(512, SP)
    nc.sync.dma_start(out=bsb[:, WK_OFF:WK_OFF + 512], in_=blob[:, WK_OFF:WK_OFF + 512])
    nc.scalar.dma_start(out=bsb[:, WQ_OFF:WQ_OFF + 512], in_=blob[:, WQ_OFF:WQ_OFF + 512])
    nc.gpsimd.dma_start(out=bsb[:, ETE_OFF:ETE_OFF + E0A],
                        in_=blob[:, ETE_OFF:ETE_OFF + E0A])
    qs = [nc.sync, nc.scalar, nc.gpsimd]
    for gi in range(len(KG)):  # all xT next: kq/scores consume it in order
        xt_dma(qs[gi % 3], gi)
    if E0A < SP:
        nc.sync.dma_start(out=bsb[:, ETE_OFF + E0A:ETE_OFF + SP],
                          in_=blob[:, ETE_OFF + E0A:ETE_OFF + SP])
    nc.sync.dma_start(out=bsb[:, WK_OFF + 512:WQ_OFF], in_=blob[:, WK_OFF + 512:WQ_OFF])
    nc.scalar.dma_start(out=bsb[:, WQ_OFF + 512:VM_OFF], in_=blob[:, WQ_OFF + 512:VM_OFF])
    nc.scalar.dma_start(out=bsb[:, :WK_OFF], in_=blob[:, :WK_OFF])  # WV
    nc.gpsimd.dma_start(out=bsb[:, VM_OFF:XT_OFF], in_=blob[:, VM_OFF:XT_OFF])
    nc.gpsimd.dma_start(out=bsb[:, ETE_OFF + SP:], in_=blob[:, ETE_OFF + SP:])

    qk = sing.tile([128, 4, SP], BF, bufs=NB, name="qk")  # planes: q0,q1,k0,k1
    vall = sing.tile([128, NKC, 65 * HPC], F32R, bufs=NB, name="vall")
    scr = sing.tile([1, 1], BF, name="scr")
    wsb = sing.tile([128, 512], BF, name="wsb")  # warmup source (zeros)
    nc.vector.memset(wsb[:, :], 0.0)  # first on DVE: no deps, runs at t=0

    # Engine preambles: observe the input DMA lanes cheaply.
    nc.vector.tensor_copy(scr, bsb[0:1, WK_OFF:WK_OFF + 1])
    nc.vector.tensor_copy(scr, bsb[0:1, XT_OFF:XT_OFF + 1])
    nc.vector.tensor_copy(scr, bsb[0:1, ETE_OFF:ETE_OFF + 1])

    if abl == "dmas":
        return

    # PE p-state warmup: the tensor engine only reaches 2.4GHz after ~3us of
    # continuous execution. Chew on garbage during the input-DMA shadow so
    # the first real matmuls run at full clock (qk is still unwritten here,
    # so these have no data dependencies). A few wide matmuls build the
    # streak; a long run of 64-wide ones keeps it alive at ~30-60ns each so
    # the first data-ready matmul is barely delayed.
    if not looped:
        wps = ps.tile([65, 512], F32, tag="hp0", bufs=1, name="warm")
        for _ in range(4):
            nc.tensor.matmul(wps[:64, :512], wsb[:, 0:64], wsb[:, 0:512],
                             start=True, stop=True)
        for _ in range(30):
            nc.tensor.matmul(wps[:64, :64], wsb[:, 0:64], wsb[:, 0:64],
                             start=True, stop=True)

    emitted_v, emitted_kq = set(), set()

    def v_proj(kc):
        if kc in emitted_v:
            return
        emitted_v.add(kc)
        pv = ps.tile([128, 3, 512], F32, tag="sc", bufs=2, name="pv")
        for f in range(4):
            base = XT_OFF + (kc * 4 + f) * 128
            nc.tensor.matmul(pv[:, 0, :256], bsb[:, base:base + 128],
                             bsb[:, WV_OFF + f * 256:WV_OFF + (f + 1) * 256],
                             start=(f == 0), stop=(f == 3))
        va = vall[:, kc, :].rearrange("p (h c) -> p h c", c=65)
        nc.vector.tensor_copy(va[:, :, 0:64],
                              pv[:, 0, :256].rearrange("p (h c) -> p h c", c=64))
        nc.vector.tensor_copy(
            va[:, :, 64:65],
            bsb[:, VM_OFF + kc * HPC:VM_OFF + (kc + 1) * HPC]
            .rearrange("p (h c) -> p h c", c=1))

    def kq_proj(cc, ch):  # cc: 0,1 = q planes; 2,3 = k planes
        if (cc, ch) in emitted_kq:
            return
        emitted_kq.add((cc, ch))
        qoff, qlen = ch
        p = ps.tile([128, 3, 512], F32, tag="sc", bufs=2, name="p")
        parts = [_xt_moving(bsb, XT_OFF, SP, f, qoff, qlen) for f in range(4)]
        for pi in range(len(parts[0])):
            for f in range(4):
                if cc < 2:
                    woff = WQ_OFF + cc * 512 + f * 128
                else:
                    woff = WK_OFF + (cc - 2) * 512 + f * 128
                loff, llen, ap = parts[f][pi]
                nc.tensor.matmul(p[:, 0, loff:loff + llen],
                                 bsb[:, woff:woff + 128], ap,
                                 start=(f == 0), stop=(f == 3))
        if cc < 2:
            ete_sl = bsb[:, ETE_OFF + cc * SP + qoff:ETE_OFF + cc * SP + qoff + qlen]
            nc.vector.tensor_add(qk[:, cc, qoff:qoff + qlen], p[:, 0, :qlen], ete_sl)
        else:
            nc.vector.tensor_copy(qk[:, cc, qoff:qoff + qlen], p[:, 0, :qlen])

    def k_chunks_for(k0, k1):
        for ch in QCH_K:
            c0, c1 = ch[0] // 128, (ch[0] + ch[1] + 127) // 128
            if c0 < k1 and c1 > k0:
                yield ch

    if abl == "proj":
        for kc in range(NKC):
            v_proj(kc)
        for cc in (2, 0, 3, 1):
            for ch in (QCH_K if cc >= 2 else QCH):
                kq_proj(cc, ch)
        return

    def sc_unit(h, ch, g):
        qoff, qlen = ch
        k0, k1 = KCG[g]
        cbase = (h % 2) * 64
        T = ps.tile([128, 3, 512], F32, tag="sc", bufs=2, name="T")
        for i, kc in enumerate(range(k0, k1)):
            nc.tensor.matmul(T[:, i, :qlen],
                             qk[cbase:cbase + 64, 2 + h // 2, kc * 128:(kc + 1) * 128],
                             qk[cbase:cbase + 64, h // 2, qoff:qoff + qlen],
                             start=True, stop=True)
        att = attp.tile([128, 3, 512], F32R, tag="att")
        fn = (mybir.ActivationFunctionType.Copy if abl == "scid"
              else mybir.ActivationFunctionType.Exp)
        nc.scalar.activation(att[:, :k1 - k0, :qlen], T[:, :k1 - k0, :qlen],
                             fn, scale=0.125)
        return att

    # Narrow remainder q-chunk: all NKC key chunks fit one PSUM bank, so a
    # single matmul batch + ONE exp replaces NG overhead-dominated ones.
    REM_OK = len(QCH) > 1 and QCH[-1][1] * NKC * 4 <= 2048

    def sc_rem_unit(h, ch):
        qoff, qlen = ch
        cbase = (h % 2) * 64
        Tr = ps.tile([128, NKC, qlen], F32, tag="hp0", bufs=1, name="Tr")
        for kc in range(NKC):
            nc.tensor.matmul(Tr[:, kc, :],
                             qk[cbase:cbase + 64, 2 + h // 2, kc * 128:(kc + 1) * 128],
                             qk[cbase:cbase + 64, h // 2, qoff:qoff + qlen],
                             start=True, stop=True)
        attr_ = attp.tile([128, NKC, qlen], F32R, tag="attr", bufs=2, name="attr")
        fn = (mybir.ActivationFunctionType.Copy if abl == "scid"
              else mybir.ActivationFunctionType.Exp)
        nc.scalar.activation(attr_[:, :, :], Tr[:, :, :], fn, scale=0.125)
        return attr_

    def pv_ci(h, ci, atts_cg, ht):
        # One PSUM bank per open accumulator (a start=True matmul resets the
        # whole bank for its partitions). hT[65, q-chunk] accumulates over
        # all NKC key chunks; att is the f32r moving operand (N>=256 keeps
        # full rate), so exp output stays f32 (bf16 ACT writes are slow).
        qoff, qlen = QCH[ci]
        ac = atts_cg[ci]
        hp = ps.tile([65, 512], F32, tag=f"hp{ci % 2}", bufs=1, name="hp")
        for kc in range(NKC):
            if isinstance(ac, tuple):  # rem-format att: [128, NKC, qlen]
                rhs = ac[1][:, kc, :qlen]
            else:
                g = next(gi for gi, (a, b) in enumerate(KCG) if a <= kc < b)
                i = kc - KCG[g][0]
                rhs = ac[g][:, i, :qlen]
            nc.tensor.matmul(hp[:, :qlen],
                             vall[:, kc, h * 65:(h + 1) * 65], rhs,
                             start=(kc == 0), stop=(kc == NKC - 1))
        nc.vector.tensor_copy(ht[:, qoff:qoff + qlen], hp[:, :qlen])

    def out_dma(h, ht, ci):
        qoff, qlen = QCH[ci]
        eng = (nc.sync, nc.gpsimd)[ci % 2]
        eng.dma_start(out=outp[h, :, qoff:qoff + qlen],
                      in_=ht[:, qoff:qoff + qlen])

    fills = iter([("v", kc) for kc in range(NKC)])
    fills2 = iter([("kq", 3, ch) for ch in QCH_K] + [("kq", 1, ch) for ch in QCH])

    def drain_fill(n):
        for _ in range(n):
            nxt = next(fills, None)
            if nxt is None:
                return
            if nxt[0] == "v":
                v_proj(nxt[1])
            else:
                kq_proj(nxt[1], nxt[2])

    prev = None  # (head, atts_cg, ht): h0's PV drains during h1's scores
    for h in range(HPC):
        atts_cg = [None] * len(QCH)
        ht = None
        if abl not in ("nopv", "scid", "attf32"):
            ht = hsb_pool.tile([65, NQC * 128], F32, tag="ht", name="ht")
        ppv = iter(range(len(QCH))) if prev is not None else iter(())

        def slot():
            if prev is not None:
                ci2 = next(ppv, None)
                if ci2 is not None:
                    pv_ci(prev[0], ci2, prev[1], prev[2])
                    out_dma(prev[0], prev[2], ci2)
                nxt = next(fills2, None)
                if nxt is not None:
                    kq_proj(nxt[1], nxt[2])
            else:
                drain_fill(2)

        # g-major over the full-width q chunks: while PE runs the JIT
        # projection chain for the next kc-group, ACT always has another
        # ready (ci, g) exp — closes the h0 DMA-paced gaps.
        ncis = len(QCH) - (1 if REM_OK else 0)
        for ci in range(ncis):
            atts_cg[ci] = [None] * NG
        for g in range(NG):
            k0, k1 = KCG[g]
            for ci in range(ncis):
                ch = QCH[ci]
                kq_proj(h // 2, ch)
                for ck in k_chunks_for(k0, k1):
                    kq_proj(2 + h // 2, ck)
                atts_cg[ci][g] = sc_unit(h, ch, g)
                slot()
                if g == NG - 1 and abl not in ("nopv", "scid", "attf32") \
                        and h == HPC - 1:
                    pv_ci(h, ci, atts_cg, ht)  # last head: inline, short tail
                    out_dma(h, ht, ci)
        if REM_OK:
            ci = len(QCH) - 1
            kq_proj(h // 2, QCH[ci])
            atts_cg[ci] = ("rem", sc_rem_unit(h, QCH[ci]))
            slot()
            if abl not in ("nopv", "scid", "attf32") and h == HPC - 1:
                pv_ci(h, ci, atts_cg, ht)
                out_dma(h, ht, ci)
        if prev is not None:
            for ci2 in ppv:
                pv_ci(prev[0], ci2, prev[1], prev[2])
                out_dma(prev[0], prev[2], ci2)
        # every non-final head defers its PV into the next head's slots
        prev = ((h, atts_cg, ht)
                if (h < HPC - 1 and abl not in ("nopv", "scid", "attf32"))
                else None)


def _prep_core(core, SP, x, etype_emb, mask, Wq, bq, Wk, bk, Wv, bv):
    NKC = SP // 128
    WV_OFF, WK_OFF, WQ_OFF, VM_OFF, XT_OFF, ETE_OFF, COLS = _offsets(SP)
    b, hg = core // 2, core % 2
    c0 = hg * CPC
    idx = np.where(mask[b] == 1)[0]
    Su = len(idx)

    blob = np.zeros((128, COLS), ml_dtypes.bfloat16)
    xs = np.zeros((SP, F), np.float32)
    xs[:Su] = x[b][idx]
    xT = xs.T
    xtb = xT.reshape(4, 128, NKC, 128).transpose(1, 2, 0, 3).reshape(128, NKC * 512)
    blob[:, XT_OFF:XT_OFF + 4 * SP] = xtb
    for f in range(4):
        blob[:, WV_OFF + f * 256:WV_OFF + (f + 1) * 256] = Wv[f * 128:(f + 1) * 128, c0:c0 + CPC]
        for half in range(2):
            cl, cr = c0 + half * 128, c0 + half * 128 + 128
            blob[:, WK_OFF + half * 512 + f * 128:WK_OFF + half * 512 + (f + 1) * 128] = \
                Wk[f * 128:(f + 1) * 128, cl:cr]
            blob[:, WQ_OFF + half * 512 + f * 128:WQ_OFF + half * 512 + (f + 1) * 128] = \
                Wq[f * 128:(f + 1) * 128, cl:cr]
    et = np.zeros((SP, CPC), np.float32)
    et[:Su] = etype_emb[b][idx][:, c0:c0 + CPC] + bq[c0:c0 + CPC]
    etT = et.T
    blob[:, ETE_OFF:ETE_OFF + SP] = etT[:128]
    blob[:, ETE_OFF + SP:ETE_OFF + 2 * SP] = etT[128:]
    pos = np.arange(128)[:, None] + 128 * np.arange(NKC)[None, :]
    vm = (pos < Su).astype(np.float32)
    blob[:, VM_OFF:VM_OFF + NKC * HPC] = np.repeat(vm, HPC, axis=1)

    return {"blob": blob}, idx


def kernel(x, etype_emb, mask, Wq, bq, Wk, bk, Wv, bv):
    global LAST_RESULT
    x = np.asarray(x, np.float32)
    etype_emb = np.asarray(etype_emb, np.float32)
    mask = np.asarray(mask)
    Wq, bq = np.asarray(Wq, np.float32), np.asarray(bq, np.float32)
    Wk, bk = np.asarray(Wk, np.float32), np.asarray(bk, np.float32)
    Wv, bv = np.asarray(Wv, np.float32), np.asarray(bv, np.float32)

    counts = [int((mask[b] == 1).sum()) for b in range(B)]
    SPL = max(2, max(counts))
    SPL += SPL % 2
    SP = max(128, ((SPL + 127) // 128) * 128)
    NQC = (SPL + 127) // 128

    nc = _build(SP, SPL=SPL)
    in_maps, idxs = [], []
    for core in range(NCORES):
        m, idx = _prep_core(core, SP, x, etype_emb, mask, Wq, bq, Wk, bk, Wv, bv)
        in_maps.append(m)
        idxs.append(idx)

    # The NTFF trace path needs antenv.axon_hooks, which this container does
    # not ship; make sure a stray BASS_TRACE=1 cannot route us into it.
    os.environ.setdefault("BASS_NEVER_TRACE", "1")
    res = run_bass_kernel_spmd(nc, in_maps, list(range(NCORES)))
    LAST_RESULT = res

    out = np.zeros((B, S, E), np.float32)
    for core in range(NCORES):
        b, hg = core // 2, core % 2
        idx = idxs[core]
        if not len(idx):
            continue
        shard = res.results[core]["out"]  # [HPC, 65, NQC*128]
        for h in range(HPC):
            num = shard[h, :64, :len(idx)]
            den = shard[h, 64, :len(idx)]
            c0 = hg * CPC + h * 64
            out[b][idx, c0:c0 + 64] = (num / den).T + bv[c0:c0 + 64]
    return out


# revision 35
# speedup vs baseline: 1.4805x; 1.4805x over previous
"""Masked multi-head attention on 8 TRN2 NeuronCores.

Sharding: core = (batch b, head-group hg). Each core computes the attention
output for one batch element and 4 of the 8 heads (a 256-wide column slice
of E). Rows with mask==0 are dropped host-side: masked queries produce
all-zero output rows, and masked keys are excluded by zeroing both their v
rows (x rows are zero) and the denominator ones-column, so no exp bias is
needed. bq is folded into etype_emb host-side; bk drops out of softmax
(per-query constant shift); bv is added host-side after the divide.

Everything bf16 on chip except PSUM accumulation and the f32 output.

Per-core pipeline, ACT(exp)-bound by design:
  qT/kT = W.T @ xT          (E-cols on partitions, S free; ete added to qT)
  v     = xT.T @ Wv         (S on partitions, DH free) + masked ones column
  per head h, kc-group g (3 key chunks), q-chunk (<=512):
    sT   = kT.T @ qT  into psum [128, 3, 512]  (keys on partitions)
    att  = exp(sT/8)  one wide ACT op per (h, q-chunk, g)
    hT[j] += att[:, j128].T @ v_aug   (transposed PV: 65-wide, full PE util)
  out[h] = hT (+den col) DMA'd per head; host transposes and divides
"""

import os

import ml_dtypes
import numpy as np

import concourse.bacc as bacc
import concourse.tile as tile
from concourse import mybir
from concourse.bass_utils import run_bass_kernel_spmd

BF = mybir.dt.bfloat16
F32 = mybir.dt.float32
F32R = mybir.dt.float32r  # f32 storage, single-pass PE mode (full rate at N>=256)

B, S, F, E, H = 4, 2048, 512, 512, 8
DH = 64
NCORES = 8
HPC = 4            # heads per core
CPC = HPC * DH     # output columns per core

LAST_RESULT = None  # BassKernelResults of the most recent run (for test harness)


def _qchunks(SPL):
    out, off = [], 0
    while off < SPL:
        ln = min(512, SPL - off)
        out.append((off, ln))
        off += ln
    return out


def _offsets(SP):
    NKC = SP // 128
    WV_OFF = 0
    WK_OFF = 1024
    WQ_OFF = 2048
    VM_OFF = 3072                    # [128, NKC*HPC] pad-aware v ones-column
    XT_OFF = VM_OFF + NKC * HPC
    ETE_OFF = XT_OFF + 4 * SP
    COLS = ETE_OFF + 2 * SP
    return WV_OFF, WK_OFF, WQ_OFF, VM_OFF, XT_OFF, ETE_OFF, COLS


def _build(SP, loop_reps=None, abl="full", SPL=None):
    if SPL is None:
        SPL = SP
    NKC = SP // 128
    NQC = (SPL + 127) // 128
    WV_OFF, WK_OFF, WQ_OFF, VM_OFF, XT_OFF, ETE_OFF, COLS = _offsets(SP)

    nc = bacc.Bacc()
    blob = nc.declare_dram_parameter("blob", [128, COLS], BF, isOutput=False)
    outp = nc.declare_dram_parameter("out", [HPC, 65, NQC * 128], F32, isOutput=True)

    with tile.TileContext(nc) as tc:
        with (
            tc.tile_pool(name="sing", bufs=1) as sing,
            tc.tile_pool(name="hsb", bufs=2) as hsb_pool,
            tc.tile_pool(name="attp", bufs=12) as attp,
            tc.tile_pool(name="ps", bufs=2, space="PSUM") as ps,
        ):
            def _body():
                _emit(nc, SP, SPL, NKC, NQC, WV_OFF, WK_OFF, WQ_OFF, VM_OFF,
                      XT_OFF, ETE_OFF, COLS, blob, outp,
                      sing, hsb_pool, attp, ps, abl, looped=loop_reps is not None)

            if loop_reps is None:
                _body()
            else:
                with tc.For_i(0, loop_reps, 1):
                    _body()
    nc.compile()
    return nc


def _xt_moving(bsb, XT_OFF, SP, f, qoff, qlen):
    """Moving-operand APs over the kc-major xT layout for q range [qoff, qoff+qlen)."""
    view = bsb[:, XT_OFF:XT_OFF + 4 * SP].rearrange("p (kc f c) -> p kc f c", f=4, c=128)
    out = []
    kc0, nfull, rem = qoff // 128, qlen // 128, qlen % 128
    if nfull:
        out.append((0, nfull * 128, view[:, kc0:kc0 + nfull, f, :]))
    if rem:
        out.append((nfull * 128, rem, view[:, kc0 + nfull, f, :rem]))
    return out


def _emit(nc, SP, SPL, NKC, NQC, WV_OFF, WK_OFF, WQ_OFF, VM_OFF, XT_OFF,
          ETE_OFF, COLS, blob, outp, sing, hsb_pool, attp, ps, abl="full",
          looped=False):
    QCH = _qchunks(SPL)     # q chunks (proj + scores)
    # k-proj chunks: 256 wide, aligned with the 2-kc xT DMA groups so a
    # scores unit never waits on xT beyond its own key range.
    QCH_K = []
    off = 0
    while off < SP:
        ln = min(256, SP - off)
        QCH_K.append((off, ln))
        off += ln
    KCG = [(i, min(i + 3, NKC)) for i in range(0, NKC, 3)]
    NG = len(KCG)

    NB = 2 if looped else 1
    bsb = sing.tile([128, COLS], BF, bufs=NB, name="bsb")
    KGS = 2  # kc chunks per xT DMA group
    KG = [(i, min(i + KGS, NKC)) for i in range(0, NKC, KGS)]

    # Input DMA over three queues (SP + ACT HWDGE rings, gpsimd SWDGE),
    # ordered so the attention front (WK/WQ/ete0/early xT) lands first.
    def xt_dma(eng, gi):
        k0, k1 = KG[gi]
        c0, c1 = XT_OFF + k0 * 512, XT_OFF + k1 * 512
        eng.dma_start(out=bsb[:, c0:c1], in_=blob[:, c0:c1])

    # Global issue order puts the scores-critical front (WK, WQ, ete[:512],
    # xT g0/g1) first; the model serializes DMA engines, so issue order is
    # landing order.
    E0A = min(512, SP)
    nc.sync.dma_start(out=bsb[:, WK_OFF:WK_OFF + 512], in_=blob[:, WK_OFF:WK_OFF + 512])
    nc.scalar.dma_start(out=bsb[:, WQ_OFF:WQ_OFF + 512], in_=blob[:, WQ_OFF:WQ_OFF + 512])
    nc.gpsimd.dma_start(out=bsb[:, ETE_OFF:ETE_OFF + E0A],
                        in_=blob[:, ETE_OFF:ETE_OFF + E0A])
    qs = [nc.sync, nc.scalar, nc.gpsimd]
    for gi in range(len(KG)):  # all xT next: kq/scores consume it in order
        xt_dma(qs[gi % 3], gi)
    if E0A < SP:
        nc.sync.dma_start(out=bsb[:, ETE_OFF + E0A:ETE_OFF + SP],
                          in_=blob[:, ETE_OFF + E0A:ETE_OFF + SP])
    nc.sync.dma_start(out=bsb[:, WK_OFF + 512:WQ_OFF], in_=blob[:, WK_OFF + 512:WQ_OFF])
    nc.scalar.dma_start(out=bsb[:, WQ_OFF + 512:VM_OFF], in_=blob[:, WQ_OFF + 512:VM_OFF])
    nc.scalar.dma_start(out=bsb[:, :WK_OFF], in_=blob[:, :WK_OFF])  # WV
    nc.gpsimd.dma_start(out=bsb[:, VM_OFF:XT_OFF], in_=blob[:, VM_OFF:XT_OFF])
    nc.gpsimd.dma_start(out=bsb[:, ETE_OFF + SP:], in_=blob[:, ETE_OFF + SP:])

    qk = sing.tile([128, 4, SP], BF, bufs=NB, name="qk")  # planes: q0,q1,k0,k1
    vall = sing.tile([128, NKC, 65 * HPC], F32R, bufs=NB, name="vall")
    scr = sing.tile([1, 1], BF, name="scr")
    wsb = sing.tile([128, 512], BF, name="wsb")  # warmup source (zeros)
    nc.vector.memset(wsb[:, :], 0.0)  # first on DVE: no deps, runs at t=0

    # Engine preambles: observe the input DMA lanes cheaply.
    nc.vector.tensor_copy(scr, bsb[0:1, WK_OFF:WK_OFF + 1])
    nc.vector.tensor_copy(scr, bsb[0:1, XT_OFF:XT_OFF + 1])
    nc.vector.tensor_copy(scr, bsb[0:1, ETE_OFF:ETE_OFF + 1])

    if abl == "dmas":
        return

    # PE p-state warmup: the tensor engine only reaches 2.4GHz after ~3us of
    # continuous execution. Chew on garbage during the input-DMA shadow so
    # the first real matmuls run at full clock (qk is still unwritten here,
    # so these have no data dependencies). A few wide matmuls build the
    # streak; a long run of 64-wide ones keeps it alive at ~30-60ns each so
    # the first data-ready matmul is barely delayed.
    if not looped:
        wps = ps.tile([65, 512], F32, tag="hp0", bufs=1, name="warm")
        for _ in range(4):
            nc.tensor.matmul(wps[:64, :512], wsb[:, 0:64], wsb[:, 0:512],
                             start=True, stop=True)
        for _ in range(30):
            nc.tensor.matmul(wps[:64, :64], wsb[:, 0:64], wsb[:, 0:64],
                             start=True, stop=True)

    emitted_v, emitted_kq = set(), set()

    def v_proj(kc):
        if kc in emitted_v:
            return
        emitted_v.add(kc)
        pv = ps.tile([128, 3, 512], F32, tag="sc", bufs=2, name="pv")
        for f in range(4):
            base = XT_OFF + (kc * 4 + f) * 128
            nc.tensor.matmul(pv[:, 0, :256], bsb[:, base:base + 128],
                             bsb[:, WV_OFF + f * 256:WV_OFF + (f + 1) * 256],
                             start=(f == 0), stop=(f == 3))
        va = vall[:, kc, :].rearrange("p (h c) -> p h c", c=65)
        nc.vector.tensor_copy(va[:, :, 0:64],
                              pv[:, 0, :256].rearrange("p (h c) -> p h c", c=64))
        nc.vector.tensor_copy(
            va[:, :, 64:65],
            bsb[:, VM_OFF + kc * HPC:VM_OFF + (kc + 1) * HPC]
            .rearrange("p (h c) -> p h c", c=1))

    def kq_proj(cc, ch):  # cc: 0,1 = q planes; 2,3 = k planes
        if (cc, ch) in emitted_kq:
            return
        emitted_kq.add((cc, ch))
        qoff, qlen = ch
        p = ps.tile([128, 3, 512], F32, tag="sc", bufs=2, name="p")
        parts = [_xt_moving(bsb, XT_OFF, SP, f, qoff, qlen) for f in range(4)]
        for pi in range(len(parts[0])):
            for f in range(4):
                if cc < 2:
                    woff = WQ_OFF + cc * 512 + f * 128
                else:
                    woff = WK_OFF + (cc - 2) * 512 + f * 128
                loff, llen, ap = parts[f][pi]
                nc.tensor.matmul(p[:, 0, loff:loff + llen],
                                 bsb[:, woff:woff + 128], ap,
                                 start=(f == 0), stop=(f == 3))
        if cc < 2:
            ete_sl = bsb[:, ETE_OFF + cc * SP + qoff:ETE_OFF + cc * SP + qoff + qlen]
            nc.vector.tensor_add(qk[:, cc, qoff:qoff + qlen], p[:, 0, :qlen], ete_sl)
        else:
            nc.vector.tensor_copy(qk[:, cc, qoff:qoff + qlen], p[:, 0, :qlen])

    def k_chunks_for(k0, k1):
        for ch in QCH_K:
            c0, c1 = ch[0] // 128, (ch[0] + ch[1] + 127) // 128
            if c0 < k1 and c1 > k0:
                yield ch

    if abl == "proj":
        for kc in range(NKC):
            v_proj(kc)
        for cc in (2, 0, 3, 1):
            for ch in (QCH_K if cc >= 2 else QCH):
                kq_proj(cc, ch)
        return

    def sc_unit(h, ch, g):
        qoff, qlen = ch
        k0, k1 = KCG[g]
        cbase = (h % 2) * 64
        T = ps.tile([128, 3, 512], F32, tag="sc", bufs=2, name="T")
        for i, kc in enumerate(range(k0, k1)):
            nc.tensor.matmul(T[:, i, :qlen],
                             qk[cbase:cbase + 64, 2 + h // 2, kc * 128:(kc + 1) * 128],
                             qk[cbase:cbase + 64, h // 2, qoff:qoff + qlen],
                             start=True, stop=True)
        att = attp.tile([128, 3, 512], F32R, tag="att")
        fn = (mybir.ActivationFunctionType.Copy if abl == "scid"
              else mybir.ActivationFunctionType.Exp)
        nc.scalar.activation(att[:, :k1 - k0, :qlen], T[:, :k1 - k0, :qlen],
                             fn, scale=0.125)
        return att

    # Narrow remainder q-chunk: all NKC key chunks fit one PSUM bank, so a
    # single matmul batch + ONE exp replaces NG overhead-dominated ones.
    REM_OK = len(QCH) > 1 and QCH[-1][1] * NKC * 4 <= 2048

    def sc_rem_unit(h, ch):
        qoff, qlen = ch
        cbase = (h % 2) * 64
        Tr = ps.tile([128, NKC, qlen], F32, tag="hp0", bufs=1, name="Tr")
        for kc in range(NKC):
            nc.tensor.matmul(Tr[:, kc, :],
                             qk[cbase:cbase + 64, 2 + h // 2, kc * 128:(kc + 1) * 128],
                             qk[cbase:cbase + 64, h // 2, qoff:qoff + qlen],
                             start=True, stop=True)
        attr_ = attp.tile([128, NKC, qlen], F32R, tag="attr", bufs=2, name="attr")
        fn = (mybir.ActivationFunctionType.Copy if abl == "scid"
              else mybir.ActivationFunctionType.Exp)
        nc.scalar.activation(attr_[:, :, :], Tr[:, :, :], fn, scale=0.125)
        return attr_

    def pv_ci(h, ci, atts_cg, ht):
        # One PSUM bank per open accumulator (a start=True matmul resets the
        # whole bank for its partitions). hT[65, q-chunk] accumulates over
        # all NKC key chunks; att is the f32r moving operand (N>=256 keeps
        # full rate), so exp output stays f32 (bf16 ACT writes are slow).
        qoff, qlen = QCH[ci]
        ac = atts_cg[ci]
        hp = ps.tile([65, 512], F32, tag=f"hp{ci % 2}", bufs=1, name="hp")
        for kc in range(NKC):
            if isinstance(ac, tuple):  # rem-format att: [128, NKC, qlen]
                rhs = ac[1][:, kc, :qlen]
            else:
                g = next(gi for gi, (a, b) in enumerate(KCG) if a <= kc < b)
                i = kc - KCG[g][0]
                rhs = ac[g][:, i, :qlen]
            nc.tensor.matmul(hp[:, :qlen],
                             vall[:, kc, h * 65:(h + 1) * 65], rhs,
                             start=(kc == 0), stop=(kc == NKC - 1))
        nc.vector.tensor_copy(ht[:, qoff:qoff + qlen], hp[:, :qlen])

    def out_dma(h, ht, ci):
        qoff, qlen = QCH[ci]
        eng = (nc.sync, nc.gpsimd)[ci % 2]
        eng.dma_start(out=outp[h, :, qoff:qoff + qlen],
                      in_=ht[:, qoff:qoff + qlen])

    fills = iter([("v", kc) for kc in range(NKC)])
    fills2 = iter([("kq", 3, ch) for ch in QCH_K] + [("kq", 1, ch) for ch in QCH])

    def drain_fill(n):
        for _ in range(n):
            nxt = next(fills, None)
            if nxt is None:
                return
            if nxt[0] == "v":
                v_proj(nxt[1])
            else:
                kq_proj(nxt[1], nxt[2])

    prev = None  # (head, atts_cg, ht): h0's PV drains during h1's scores
    for h in range(HPC):
        atts_cg = [None] * len(QCH)
        ht = None
        if abl not in ("nopv", "scid", "attf32"):
            ht = hsb_pool.tile([65, NQC * 128], F32, tag="ht", name="ht")
        ppv = iter(range(len(QCH))) if prev is not None else iter(())

        slot_i = [0]

        def slot():
            if prev is not None:
                ci2 = next(ppv, None)
                if ci2 is not None:
                    pv_ci(prev[0], ci2, prev[1], prev[2])
                    out_dma(prev[0], prev[2], ci2)
                nxt = next(fills2, None)
                if nxt is not None:
                    kq_proj(nxt[1], nxt[2])
            else:
                # backload h0's fills: early slots feed ACT, later absorb
                drain_fill(1 if slot_i[0] < 3 else 3)
                slot_i[0] += 1

        # g-major over the full-width q chunks: while PE runs the JIT
        # projection chain for the next kc-group, ACT always has another
        # ready (ci, g) exp — closes the h0 DMA-paced gaps.
        ncis = len(QCH) - (1 if REM_OK else 0)
        for ci in range(ncis):
            atts_cg[ci] = [None] * NG
        for g in range(NG):
            k0, k1 = KCG[g]
            for ci in range(ncis):
                ch = QCH[ci]
                kq_proj(h // 2, ch)
                for ck in k_chunks_for(k0, k1):
                    kq_proj(2 + h // 2, ck)
                atts_cg[ci][g] = sc_unit(h, ch, g)
                slot()
                if g == NG - 1 and abl not in ("nopv", "scid", "attf32") \
                        and h == HPC - 1:
                    pv_ci(h, ci, atts_cg, ht)  # last head: inline, short tail
                    out_dma(h, ht, ci)
        if REM_OK:
            ci = len(QCH) - 1
            kq_proj(h // 2, QCH[ci])
            atts_cg[ci] = ("rem", sc_rem_unit(h, QCH[ci]))
            slot()
            if abl not in ("nopv", "scid", "attf32") and h == HPC - 1:
                pv_ci(h, ci, atts_cg, ht)
                out_dma(h, ht, ci)
        if prev is not None:
            for ci2 in ppv:
                pv_ci(prev[0], ci2, prev[1], prev[2])
                out_dma(prev[0], prev[2], ci2)
        # every non-final head defers its PV into the next head's slots
        prev = ((h, atts_cg, ht)
                if (h < HPC - 1 and abl not in ("nopv", "scid", "attf32"))
                else None)


def _prep_core(core, SP, x, etype_emb, mask, Wq, bq, Wk, bk, Wv, bv):
    NKC = SP // 128
    WV_OFF, WK_OFF, WQ_OFF, VM_OFF, XT_OFF, ETE_OFF, COLS = _offsets(SP)
    b, hg = core // 2, core % 2
    c0 = hg * CPC
    idx = np.where(mask[b] == 1)[0]
    Su = len(idx)

    blob = np.zeros((128, COLS), ml_dtypes.bfloat16)
    xs = np.zeros((SP, F), np.float32)
    xs[:Su] = x[b][idx]
    xT = xs.T
    xtb = xT.reshape(4, 128, NKC, 128).transpose(1, 2, 0, 3).reshape(128, NKC * 512)
    blob[:, XT_OFF:XT_OFF + 4 * SP] = xtb
    for f in range(4):
        blob[:, WV_OFF + f * 256:WV_OFF + (f + 1) * 256] = Wv[f * 128:(f + 1) * 128, c0:c0 + CPC]
        for half in range(2):
            cl, cr = c0 + half * 128, c0 + half * 128 + 128
            blob[:, WK_OFF + half * 512 + f * 128:WK_OFF + half * 512 + (f + 1) * 128] = \
                Wk[f * 128:(f + 1) * 128, cl:cr]
            blob[:, WQ_OFF + half * 512 + f * 128:WQ_OFF + half * 512 + (f + 1) * 128] = \
                Wq[f * 128:(f + 1) * 128, cl:cr]
    et = np.zeros((SP, CPC), np.float32)
    et[:Su] = etype_emb[b][idx][:, c0:c0 + CPC] + bq[c0:c0 + CPC]
    etT = et.T
    blob[:, ETE_OFF:ETE_OFF + SP] = etT[:128]
    blob[:, ETE_OFF + SP:ETE_OFF + 2 * SP] = etT[128:]
    pos = np.arange(128)[:, None] + 128 * np.arange(NKC)[None, :]
    vm = (pos < Su).astype(np.float32)
    blob[:, VM_OFF:VM_OFF + NKC * HPC] = np.repeat(vm, HPC, axis=1)

    return {"blob": blob}, idx


def kernel(x, etype_emb, mask, Wq, bq, Wk, bk, Wv, bv):
    global LAST_RESULT
    x = np.asarray(x, np.float32)
    etype_emb = np.asarray(etype_emb, np.float32)
    mask = np.asarray(mask)
    Wq, bq = np.asarray(Wq, np.float32), np.asarray(bq, np.float32)
    Wk, bk = np.asarray(Wk, np.float32), np.asarray(bk, np.float32)
    Wv, bv = np.asarray(Wv, np.float32), np.asarray(bv, np.float32)

    counts = [int((mask[b] == 1).sum()) for b in range(B)]
    SPL = max(2, max(counts))
    SPL += SPL % 2
    SP = max(128, ((SPL + 127) // 128) * 128)
    NQC = (SPL + 127) // 128

    nc = _build(SP, SPL=SPL)
    in_maps, idxs = [], []
    for core in range(NCORES):
        m, idx = _prep_core(core, SP, x, etype_emb, mask, Wq, bq, Wk, bk, Wv, bv)
        in_maps.append(m)
        idxs.append(idx)

    # The NTFF trace path needs antenv.axon_hooks, which this container does
    # not ship; make sure a stray BASS_TRACE=1 cannot route us into it.
    os.environ.setdefault("BASS_NEVER_TRACE", "1")
    res = run_bass_kernel_spmd(nc, in_maps, list(range(NCORES)))
    LAST_RESULT = res

    out = np.zeros((B, S, E), np.float32)
    for core in range(NCORES):
        b, hg = core // 2, core % 2
        idx = idxs[core]
        if not len(idx):
            continue
        shard = res.results[core]["out"]  # [HPC, 65, NQC*128]
        for h in range(HPC):
            num = shard[h, :64, :len(idx)]
            den = shard[h, 64, :len(idx)]
            c0 = hg * CPC + h * 64
            out[b][idx, c0:c0 + 64] = (num / den).T + bv[c0:c0 + 64]
    return out
